# revision 9
# baseline (speedup 1.0000x reference)
"""GGNN (gated graph NN) forward on 8 Trainium2 NeuronCores.

Strategy (node-partitioned, SPMD — one Bass program, per-core data):
  - Nodes are permuted and packed into 8 cores x 20 bins x 96 node-column
    slots such that, for every (bin, etype), the number of in-edges is <= 128.
    This makes the aggregation a fixed static structure: one 128-edge tile per
    (etype, bin).
  - h is kept per-core transposed in SBUF ([128 hid, 2048 node-cols]) for all
    dense matmuls; a row-major bf16 copy lives in DRAM (AllGathered each
    step) and is the source for per-edge dma_gather.
  - Per-edge gathers use PREPARE_ONLY descriptor generation + trigger_dma:
    the Q7 (GpSimd) descriptor-gen work (~2.5us per 896-row call, ~98us/step)
    for step s+1 runs during step s's compute/collective phases, off the
    critical path.  Each step fires 4 triggers (one per SWDGE queue).
    Gather buffers rotate over 8 tiles (2-step parity x 4 etypes) so preps
    never wait on current-step consumers.
  - Per step, per etype t: per bin, a one-hot/count matrix S (host-built,
    bf16, exact) scatter-sums the gathered rows into B_t[d, dst] in PSUM via
    TensorE matmuls; then aT += W_t^T @ B_t (float32r).  GRU update runs
    fully on-chip (gates in PSUM, sigmoid/tanh on ACT with per-partition
    bias); graph readout is a one-hot matmul + AllReduce.
"""

import numpy as np
import ml_dtypes

import concourse.bacc as bacc
import concourse.mybir as mybir
import concourse.tile as tile
from concourse.masks import make_identity
from concourse.bass_utils import run_bass_kernel_spmd

BF16_NP = ml_dtypes.bfloat16

F32 = mybir.dt.float32
F32R = mybir.dt.float32r
BF16 = mybir.dt.bfloat16
I16 = mybir.dt.int16
AF = mybir.ActivationFunctionType
OP = mybir.AluOpType

HID = 128
USE_F32R = True  # fp32r (tf32-ish) for W/GRU matmuls
PREP_TRIGGER = False  # SWDGE prepare_only + trigger_dma gathers (slower: prep
                      # desc-gen is ~1.7x the cost and Tile adds a ~1.6us
                      # InstIncSwdgeSem per prep)
SPLIT_AG = True  # two range-AllGathers per step, first hidden under GRU tail
AG_SPLIT = 1024  # local rows in the first AllGather (512-aligned)


class Cfg:
    def __init__(self, n_cores, shard, bin_cols, n_etypes, n_steps, n_graphs,
                 n_classes, in_dim):
        assert shard % 128 == 0 and shard % bin_cols == 0
        self.n_cores = n_cores
        self.shard = shard                  # node slots per core
        self.bin = bin_cols                 # node columns per bin
        self.bins = shard // bin_cols       # bins per core
        assert self.bins % 2 == 0
        self.half_bins = self.bins // 2
        self.half_cols = self.half_bins * bin_cols   # node cols per B-half
        assert self.half_cols <= 1024
        self.ntot = n_cores * shard
        self.vpad = ((shard + 511) // 512) * 512     # aT psum width
        self.jt = shard // 128              # 128-wide transpose tiles per core
        self.T = n_etypes
        self.steps = n_steps
        self.G = n_graphs
        self.C = n_classes
        self.in_dim = in_dim
        self.idxc = self.bins * 128 // 16   # idx cols per etype
        self.gbins = 7                      # bins per dma_gather call
        self.scols = self.T * self.bins * self.bin   # S cols total
        self.pw = 128                       # gather row elems (bf16)


CFG_FULL = dict(n_cores=8, shard=1920, bin_cols=96, n_etypes=13, n_steps=6,
                n_graphs=64, n_classes=10, in_dim=100)


# ---------------------------------------------------------------- host prep

def _pack_nodes(deg, cfg, rng_order=None):
    """Assign each node to a (global bin, slot) s.t. per-(bin,etype) in-edge
    count <= 128 and per-bin node count <= cfg.bin. Returns slot_of[node]."""
    N = deg.shape[0]
    nbins = cfg.n_cores * cfg.bins
    assert N <= nbins * cfg.bin, "not enough node slots"
    used_e = np.zeros((nbins, cfg.T), np.int64)
    used_s = np.zeros(nbins, np.int64)
    order = np.lexsort((-deg.sum(1), -deg.max(1)))
    bin_of = np.empty(N, np.int64)
    for v in order:
        dv = deg[v]
        ok = (used_s < cfg.bin) & ((used_e + dv) <= 128).all(1)
        if not ok.any():
            raise RuntimeError("bin packing failed; reduce bin_cols")
        cand = np.nonzero(ok)[0]
        load = (used_e[cand] + dv).max(1) * 1.0 + used_s[cand] * 0.01
        b = cand[np.argmin(load)]
        used_e[b] += dv
        bin_of[v] = b
        used_s[b] += 1
    # slots within each bin in node order
    slot_of = np.empty(N, np.int64)
    fill = np.zeros(nbins, np.int64)
    for v in range(N):
        b = bin_of[v]
        core, lb = b // cfg.bins, b % cfg.bins
        slot_of[v] = core * cfg.shard + lb * cfg.bin + fill[b]
        fill[b] += 1
    return slot_of


def make_plan(feat, src, dst, etypes, graph_ids, W_e, b_e, W_ih, W_hh, b_ih,
              b_hh, W_cls, b_cls, cfg):
    N = feat.shape[0]
    T, S_, B_, BINS = cfg.T, cfg.shard, cfg.bin, cfg.bins
    deg = np.zeros((N, T), np.int64)
    np.add.at(deg, (dst, etypes), 1)
    slot_of = _pack_nodes(deg, cfg)

    # --- edge plan ---
    dslot = slot_of[dst]
    sslot = slot_of[src]
    gbin = dslot // B_                       # global bin (slot space is bin-aligned)
    core = dslot // S_
    lbin = gbin - core * BINS
    tile_id = etypes.astype(np.int64) * BINS + lbin      # per-core tile index
    order = np.lexsort((dslot, tile_id, core))
    c_o, t_o, ds_o, ss_o = core[order], tile_id[order], dslot[order], sslot[order]
    # row index within each (core, tile) group
    key = c_o * (T * BINS) + t_o
    boundaries = np.nonzero(np.diff(key))[0] + 1
    starts = np.concatenate([[0], boundaries])
    group_of = np.searchsorted(starts, np.arange(len(key)), side="right") - 1
    row = np.arange(len(key)) - starts[group_of]
    assert row.max() < 128, "edge cap exceeded (packing bug)"

    NC = cfg.n_cores
    # gather-row renumbering: the collective source buffer is laid out as
    # [all cores' rows 0..AG_SPLIT) | all cores' rows [AG_SPLIT..shard)] so
    # each of the two range-AllGathers writes one contiguous slice.
    def g_row(slot):
        c, j = slot // S_, slot % S_
        if not SPLIT_AG:
            return slot
        lo = j < AG_SPLIT
        return np.where(lo, c * AG_SPLIT + j,
                        NC * AG_SPLIT + c * (S_ - AG_SPLIT) + (j - AG_SPLIT))

    S_host = np.zeros((NC, 128, cfg.scols), np.float32)
    idx_lin = np.zeros((NC, T * BINS, 128), np.int64)
    np.add.at(S_host, (c_o, row, t_o * B_ + (ds_o % B_)), 1.0)
    idx_lin[c_o, t_o, row] = g_row(ss_o)

    # idx wrap: per etype block of bins*128 positions -> [16, idxc]
    idx_lin = idx_lin.reshape(NC, T, BINS * 128)
    wrapped = idx_lin.reshape(NC, T, cfg.idxc, 16).transpose(0, 3, 1, 2)
    idx_host = np.tile(wrapped.reshape(NC, 16, T * cfg.idxc), (1, 8, 1))
    idx_host = np.ascontiguousarray(idx_host).astype(np.int16)

    # --- degree matrix (for b_e bias), per core [T, vpad] ---
    D_host = np.zeros((NC, T, cfg.vpad), np.float32)
    np.add.at(D_host, (core, etypes.astype(np.int64), dslot % S_), 1.0)

    # --- graph one-hot, per core [128, jt*G] ---
    node_of_slot = np.full(cfg.ntot, -1, np.int64)
    node_of_slot[slot_of] = np.arange(N)
    G_host = np.zeros((NC, 128, cfg.jt * cfg.G), np.float32)
    for c in range(NC):
        sl = node_of_slot[c * S_:(c + 1) * S_]
        valid = np.nonzero(sl >= 0)[0]
        j, p = valid // 128, valid % 128
        g = graph_ids[sl[valid]]
        G_host[c, p, j * cfg.G + g] = 1.0

    # --- h0 ---
    h0 = np.zeros((cfg.ntot, HID), np.float32)
    h0[slot_of, :cfg.in_dim] = feat
    h0_gr = np.empty_like(h0)
    h0_gr[g_row(np.arange(cfg.ntot))] = h0
    h0_pair = h0_gr.astype(BF16_NP)
    h0T = np.zeros((NC, 128, cfg.vpad), np.float32)
    for c in range(NC):
        h0T[c, :, :S_] = h0[c * S_:(c + 1) * S_].T

    # --- weights ---
    W_host = np.ascontiguousarray(W_e.transpose(1, 0, 2).reshape(128, T * HID))
    WihT = np.ascontiguousarray(W_ih.T)             # [128, 384]
    WhhT = np.ascontiguousarray(W_hh.T)
    bias4 = np.stack([
        b_ih[0:HID] + b_hh[0:HID],                  # r
        b_ih[HID:2 * HID] + b_hh[HID:2 * HID],      # z
        b_ih[2 * HID:],                             # n (input side)
        b_hh[2 * HID:],                             # n (hidden side)
    ], axis=1).astype(np.float32)                   # [128, 4]
    WclsT = np.ascontiguousarray(W_cls.T).astype(np.float32)   # [128, C]
    bclsG = np.tile(b_cls[None, :], (cfg.G, 1)).astype(np.float32)

    in_maps = []
    for c in range(NC):
        in_maps.append({
            "h0_pair": h0_pair,
            "h0T": h0T[c],
            "S": S_host[c].astype(BF16_NP),
            "idx": idx_host[c],
            "D": D_host[c],
            "G": G_host[c],
            "W": W_host.astype(np.float32),
            "Wih": WihT.astype(np.float32),
            "Whh": WhhT.astype(np.float32),
            "be": np.ascontiguousarray(b_e).astype(np.float32),
            "bias4": bias4,
            "Wcls": WclsT,
            "bcls": bclsG,
        })
    return in_maps


# ---------------------------------------------------------------- bass build

def _window_pieces(cfg, b):
    """Split bin b's 96-col window at 512-boundaries of its B-half tile.
    Returns (half, [(b_off, width, s_off), ...]) with b_off relative to the
    half tile."""
    half = b // cfg.half_bins
    start = (b - half * cfg.half_bins) * cfg.bin
    end = start + cfg.bin
    pieces = []
    cur = start
    while cur < end:
        nxt = min(end, (cur // 512 + 1) * 512)
        pieces.append((cur, nxt - cur, cur - start))
        cur = nxt
    return half, pieces


def _wmm_pieces(cfg, half):
    """aT col ranges for the W_t matmul of one B half: split the half's node
    cols at 512-boundaries of the aT tile. Returns [(at_off, width, b_off)]."""
    lo = half * cfg.half_cols
    hi = lo + cfg.half_cols
    out = []
    cur = lo
    while cur < hi:
        nxt = min(hi, (cur // 512 + 1) * 512)
        out.append((cur, nxt - cur, cur - lo))
        cur = nxt
    return out


def build_nc(cfg):
    nc = bacc.Bacc("TRN2", target_bir_lowering=False, debug=False,
                   num_devices=cfg.n_cores, num_swdge_queues=4)
    T, BINS, B_, VP, JT = cfg.T, cfg.bins, cfg.bin, cfg.vpad, cfg.jt
    NCH = 512  # gru chunk
    NQ = 4    # SWDGE queues

    d_pair0 = nc.dram_tensor("h0_pair", [cfg.ntot, cfg.pw], BF16, kind="ExternalInput")
    d_h0T = nc.dram_tensor("h0T", [128, VP], F32, kind="ExternalInput")
    d_S = nc.dram_tensor("S", [128, cfg.scols], BF16, kind="ExternalInput")
    d_idx = nc.dram_tensor("idx", [128, T * cfg.idxc], I16, kind="ExternalInput")
    d_D = nc.dram_tensor("D", [T, VP], F32, kind="ExternalInput")
    d_G = nc.dram_tensor("G", [128, JT * cfg.G], F32, kind="ExternalInput")
    d_W = nc.dram_tensor("W", [128, T * HID], F32, kind="ExternalInput")
    d_Wih = nc.dram_tensor("Wih", [128, 3 * HID], F32, kind="ExternalInput")
    d_Whh = nc.dram_tensor("Whh", [128, 3 * HID], F32, kind="ExternalInput")
    d_be = nc.dram_tensor("be", [T, HID], F32, kind="ExternalInput")
    d_bias4 = nc.dram_tensor("bias4", [128, 4], F32, kind="ExternalInput")
    d_Wcls = nc.dram_tensor("Wcls", [128, cfg.C], F32, kind="ExternalInput")
    d_bcls = nc.dram_tensor("bcls", [cfg.G, cfg.C], F32, kind="ExternalInput")
    d_out = nc.dram_tensor("out", [cfg.G, cfg.C], F32, kind="ExternalOutput")

    # internal dram (collective bounce, double-buffered)
    aspace = "Shared" if cfg.n_cores > 4 else "Local"
    cc_in = [nc.dram_tensor(f"cc_in{i}", [cfg.shard, cfg.pw], BF16)
             for i in range(2)]
    cc_out = [nc.dram_tensor(f"cc_out{i}", [cfg.ntot, cfg.pw], BF16,
                             addr_space=aspace) for i in range(2)]
    hg_in = nc.dram_tensor("hg_in", [cfg.G, HID], F32)
    hg_out = nc.dram_tensor("hg_out", [cfg.G, HID], F32, addr_space=aspace)

    MMDT = F32R if USE_F32R else F32

    with tile.TileContext(nc) as tc:
        def sb(name, shape, dt=F32):
            return nc.alloc_sbuf_tensor(name, list(shape), dt).ap()

        def ps(name, shape, dt=F32):
            return nc.alloc_psum_tensor(name, list(shape), dt).ap()

        S_sb = sb("S_sb", [128, cfg.scols], BF16)
        idx_sb = sb("idx_sb", [128, T * cfg.idxc], I16)
        hT = sb("hT", [128, VP])
        aT_sb = sb("aT_sb", [128, VP], MMDT)
        W_sb = sb("W_sb", [128, T * HID], MMDT)
        Wih_sb = sb("Wih_sb", [128, 3 * HID], MMDT)
        Whh_sb = sb("Whh_sb", [128, 3 * HID], MMDT)
        be_sb = sb("be_sb", [T, HID], MMDT)
        D_sb = sb("D_sb", [T, VP], MMDT)
        bias_sb = sb("bias_sb", [128, 4])
        G_sb = sb("G_sb", [128, JT * cfg.G])
        Wcls_sb = sb("Wcls_sb", [128, cfg.C])
        bcls_sb = sb("bcls_sb", [cfg.G, cfg.C])
        ident = sb("ident", [128, 128])
        h_rows = sb("h_rows", [128, JT * 128])
        pair_sb = sb("pair_sb", [128, JT * cfg.pw], BF16)
        hg_sb = sb("hg_sb", [cfg.G, HID])
        hgT_sb = sb("hgT_sb", [128, cfg.G])
        out_sb = sb("out_sb", [cfg.G, cfg.C])
        hTr = sb("hTr", [128, VP], F32R) if USE_F32R else None

        NGB = 8
        gbuf = [sb(f"gbuf{i}", [128, BINS * cfg.pw], BF16) for i in range(NGB)]
        Bsb = [sb(f"Bsb{i}", [128, cfg.half_cols], MMDT) for i in range(2)]
        # GRU scratch, 2 sets alternating by chunk parity (in-place reuse)
        gsc = [{nm: sb(f"gsc{i}_{nm}", [128, NCH])
                for nm in ("r", "z", "hn", "n")}
               for i in range(2)]

        B_ps = [ps(f"B_ps{i}", [128, 1024]) for i in range(2)]
        aT_ps = ps("aT_ps", [128, VP])

        # ---------------- setup loads ----------------
        if USE_F32R:
            # load fp32 via one staging tile, round via DVE copy into f32r
            stage = sb("stage", [128, VP])
            for dsrc, dst_sb, w in ((d_W, W_sb, T * HID),
                                    (d_Wih, Wih_sb, 3 * HID),
                                    (d_Whh, Whh_sb, 3 * HID)):
                nc.sync.dma_start(stage[:, :w], dsrc[:])
                nc.vector.tensor_copy(dst_sb[:], stage[:, :w])
            stage4 = stage[0:T, 0:HID]
            nc.sync.dma_start(stage4, d_be[:])
            nc.vector.tensor_copy(be_sb[:], stage4)
            nc.sync.dma_start(stage[0:T, :], d_D[:])
            nc.vector.tensor_copy(D_sb[:], stage[0:T, :])
        else:
            nc.sync.dma_start(W_sb[:], d_W[:])
            nc.sync.dma_start(Wih_sb[:], d_Wih[:])
            nc.sync.dma_start(Whh_sb[:], d_Whh[:])
            nc.sync.dma_start(be_sb[:], d_be[:])
            nc.sync.dma_start(D_sb[:], d_D[:])
        nc.sync.dma_start(idx_sb[:], d_idx[:])
        SC = BINS * B_
        for t in range(T):
            nc.sync.dma_start(S_sb[:, t * SC:(t + 1) * SC],
                              d_S[:, t * SC:(t + 1) * SC])
        nc.sync.dma_start(hT[:], d_h0T[:])
        nc.sync.dma_start(bias_sb[:], d_bias4[:])
        nc.sync.dma_start(G_sb[:], d_G[:])
        nc.sync.dma_start(Wcls_sb[:], d_Wcls[:])
        nc.sync.dma_start(bcls_sb[:], d_bcls[:])
        make_identity(nc, ident[:])

        # ---------------- gather plan ----------------
        GB = cfg.gbins
        CALLS = [(t, b0, min(GB, BINS - b0))
                 for t in range(T) for b0 in range(0, BINS, GB)]

        def g3_of(s, t):
            g = gbuf[(s % 2) * 4 + (t % 4)]
            return g[:].rearrange("p (b d) -> p b d", d=cfg.pw)

        def emit_preps(s):
            """PREPARE_ONLY descriptor gen for all of step s's gathers,
            grouped by queue so each queue's trigger can fire as soon as its
            own preps are written."""
            pair_src = d_pair0 if s == 0 else cc_out[s % 2]
            sems = [nc.alloc_semaphore(f"gsem_{s}_{q}") for q in range(NQ)]
            for q in range(NQ):
                for i, (t, b0, nb) in enumerate(CALLS):
                    if i % NQ != q:
                        continue
                    nc.gpsimd.dma_gather(
                        g3_of(s, t)[:, b0:b0 + nb, :], pair_src[:],
                        idx_sb[:, t * cfg.idxc + b0 * 8:
                               t * cfg.idxc + (b0 + nb) * 8],
                        nb * 128, nb * 128, cfg.pw,
                        prepare_only=True, sem=sems[q], queue_num=q)

        if PREP_TRIGGER:
            emit_preps(0)

        # ---------------- steps ----------------
        gq = [0]  # rotating SWDGE queue for non-prep gathers
        for s in range(cfg.steps):
            pair_src = d_pair0 if s == 0 else cc_out[s % 2]

            if PREP_TRIGGER:
                for q in range(NQ):
                    nc.gpsimd.trigger_dma(count=None, queue_num=q)

            # deg * b_e bias: aT = be^T @ D  (start=True covers all of aT)
            for c0 in range(0, VP, 512):
                nc.tensor.matmul(aT_ps[:, c0:c0 + 512], be_sb[:],
                                 D_sb[:, c0:c0 + 512], start=True, stop=False)

            for t in range(T):
                g3 = g3_of(s, t)
                if not PREP_TRIGGER:
                    for b0 in range(0, BINS, GB):
                        nb = min(GB, BINS - b0)
                        nc.gpsimd.dma_gather(
                            g3[:, b0:b0 + nb, :], pair_src[:],
                            idx_sb[:, t * cfg.idxc + b0 * 8:
                                   t * cfg.idxc + (b0 + nb) * 8],
                            nb * 128, nb * 128, cfg.pw,
                            queue_num=gq[0] % NQ)
                        gq[0] += 1
                for half in range(2):
                    Bp = B_ps[half]
                    # flat entry list: (bank, b_off, w, s_col)
                    entries = []
                    for bi in range(cfg.half_bins):
                        b = half * cfg.half_bins + bi
                        _, pieces = _window_pieces(cfg, b)
                        sbase = (t * BINS + b) * B_
                        for (b_off, w, s_off) in pieces:
                            entries.append((b_off // 512, b_off, w,
                                            sbase + s_off))
                    first_of = {}
                    last_of = {}
                    for i, e in enumerate(entries):
                        first_of.setdefault(e[0], i)
                        last_of[e[0]] = i
                    for i, (bank, b_off, w, s_col) in enumerate(entries):
                        nc.tensor.matmul(
                            Bp[:, b_off:b_off + w],
                            g3[:, (b_off + half * cfg.half_cols) // B_, 0:HID],
                            S_sb[:, s_col:s_col + w],
                            start=(first_of[bank] == i),
                            stop=(last_of[bank] == i))
                    # PSUM -> SBUF (rounds to f32r when enabled)
                    if (t * 2 + half) % 2:
                        nc.scalar.activation(Bsb[half][:, :], Bp[:, :cfg.half_cols],
                                             AF.Identity)
                    else:
                        nc.vector.tensor_copy(Bsb[half][:, :], Bp[:, :cfg.half_cols])
                    # aT += W_t^T @ B_half.  stop=True only on the final
                    # accumulation touching each aT bank (t==T-1; for banks
                    # shared by both halves, only half 1's piece closes it).
                    lhsW = W_sb[:, t * HID:(t + 1) * HID]
                    for (at_off, w, b_off) in _wmm_pieces(cfg, half):
                        bank_end = (at_off + w - 1) // 512
                        shared = (cfg.half_cols % 512) != 0 and \
                            bank_end == cfg.half_cols // 512
                        is_stop = (t == T - 1) and not (half == 0 and shared)
                        nc.tensor.matmul(aT_ps[:, at_off:at_off + w], lhsW,
                                         Bsb[half][:, b_off:b_off + w],
                                         start=False, stop=is_stop)

            gru_rhs_h = hTr if USE_F32R else hT

            # aT psum -> sbuf (chunked; must all finish before odd GRU
            # chunks reuse aT_ps banks for gates)
            for c0 in range(0, VP, NCH):
                nc.scalar.activation(aT_sb[:, c0:c0 + NCH],
                                     aT_ps[:, c0:c0 + NCH], AF.Identity)
                if USE_F32R:
                    nc.vector.tensor_copy(hTr[:, c0:c0 + NCH], hT[:, c0:c0 + NCH])

            # ---------------- GRU ----------------
            for ci, c0 in enumerate(range(0, VP, NCH)):
                if ci % 2 == 0:
                    gA, gB = B_ps[0], B_ps[1]
                else:
                    gA, gB = aT_ps[:, 0:2 * NCH], aT_ps[:, 2 * NCH:4 * NCH]
                r_ps, z_ps = gA[:, 0:NCH], gA[:, NCH:2 * NCH]
                ni_ps, nh_ps = gB[:, 0:NCH], gB[:, NCH:2 * NCH]
                a_c = aT_sb[:, c0:c0 + NCH]
                h_c = gru_rhs_h[:, c0:c0 + NCH]
                nc.tensor.matmul(r_ps, Wih_sb[:, 0:HID], a_c, start=True, stop=False)
                nc.tensor.matmul(r_ps, Whh_sb[:, 0:HID], h_c, start=False, stop=True)
                nc.tensor.matmul(z_ps, Wih_sb[:, HID:2 * HID], a_c, start=True, stop=False)
                nc.tensor.matmul(z_ps, Whh_sb[:, HID:2 * HID], h_c, start=False, stop=True)
                nc.tensor.matmul(ni_ps, Wih_sb[:, 2 * HID:3 * HID], a_c, start=True, stop=True)
                nc.tensor.matmul(nh_ps, Whh_sb[:, 2 * HID:3 * HID], h_c, start=True, stop=True)

                sc = gsc[ci % 2]
                r_sb, z_sb, hn_sb, n_sb = sc["r"], sc["z"], sc["hn"], sc["n"]
                nc.scalar.activation(r_sb[:], r_ps, AF.Sigmoid, bias=bias_sb[:, 0:1])
                nc.scalar.activation(z_sb[:], z_ps, AF.Sigmoid, bias=bias_sb[:, 1:2])
                nc.scalar.activation(hn_sb[:], nh_ps, AF.Identity, bias=bias_sb[:, 3:4])
                nc.vector.tensor_tensor(out=r_sb[:], in0=r_sb[:], in1=hn_sb[:], op=OP.mult)
                nc.vector.tensor_tensor(out=r_sb[:], in0=r_sb[:], in1=ni_ps, op=OP.add)
                nc.scalar.activation(n_sb[:], r_sb[:], AF.Tanh, bias=bias_sb[:, 2:3])
                nc.vector.tensor_tensor(out=hn_sb[:], in0=hT[:, c0:c0 + NCH], in1=n_sb[:], op=OP.subtract)
                nc.vector.tensor_tensor(out=hn_sb[:], in0=hn_sb[:], in1=z_sb[:], op=OP.mult)
                nc.vector.tensor_tensor(out=hT[:, c0:c0 + NCH], in0=hn_sb[:], in1=n_sb[:], op=OP.add)

            # ------- transpose h -> rows; pack bf16 + DMA per 512-range ---
            tp_slots = [B_ps[0][:, 0:128], B_ps[1][:, 0:128],
                        aT_ps[:, 0:128]]
            if VP >= 2048:
                tp_slots.append(aT_ps[:, 1024:1152])
            hr3 = h_rows[:].rearrange("p (j d) -> p j d", d=128)
            pr3 = pair_sb[:].rearrange("p (j d) -> p j d", d=cfg.pw)
            dst = cc_in[(s + 1) % 2]
            dst3 = dst[:].rearrange("(j p) d -> p j d", p=128)
            ccd = cc_out[(s + 1) % 2]
            rgrp = [list(range(cfg.n_cores))]
            JA = AG_SPLIT // 128  # j-tiles in the first AllGather
            j_done = 0
            for j in range(JT):
                tp = tp_slots[j % len(tp_slots)]
                nc.tensor.transpose(tp, hT[:, j * 128:(j + 1) * 128], ident[:])
                if j % 2:
                    nc.scalar.activation(h_rows[:, j * 128:(j + 1) * 128], tp, AF.Identity)
                else:
                    nc.vector.tensor_copy(h_rows[:, j * 128:(j + 1) * 128], tp)
                rng_end = (j + 1) * 128
                if s < cfg.steps - 1 and (rng_end % NCH == 0 or j == JT - 1):
                    j0, j1 = j_done, j + 1
                    j_done = j + 1
                    nc.scalar.activation(pr3[:, j0:j1, 0:HID],
                                         hr3[:, j0:j1, :], AF.Identity)
                    nc.sync.dma_start(dst3[:, j0:j1, :], pr3[:, j0:j1, :])
                if s < cfg.steps - 1 and SPLIT_AG and j + 1 == JA:
                    nc.gpsimd.collective_compute(
                        "AllGather", OP.bypass,
                        ins=[dst[0:AG_SPLIT, :]],
                        outs=[ccd[0:cfg.n_cores * AG_SPLIT, :]],
                        replica_groups=rgrp)
            if s < cfg.steps - 1:
                if SPLIT_AG:
                    nc.gpsimd.collective_compute(
                        "AllGather", OP.bypass,
                        ins=[dst[AG_SPLIT:cfg.shard, :]],
                        outs=[ccd[cfg.n_cores * AG_SPLIT:cfg.ntot, :]],
                        replica_groups=rgrp)
                else:
                    nc.gpsimd.collective_compute(
                        "AllGather", OP.bypass,
                        ins=[dst[:]], outs=[ccd[:]],
                        replica_groups=rgrp)
                if PREP_TRIGGER:
                    emit_preps(s + 1)

        # ---------------- readout ----------------
        hg_ps = B_ps[0][0:cfg.G, 0:HID]
        for j in range(JT):
            nc.tensor.matmul(hg_ps, G_sb[:, j * cfg.G:(j + 1) * cfg.G],
                             h_rows[:, j * 128:(j + 1) * 128],
                             start=(j == 0), stop=(j == JT - 1))
        nc.scalar.activation(hg_sb[:], hg_ps, AF.Identity)
        nc.sync.dma_start(hg_in[:], hg_sb[:])
        nc.gpsimd.collective_compute(
            "AllReduce", OP.add, ins=[hg_in[:]], outs=[hg_out[:]],
            replica_groups=[list(range(cfg.n_cores))])
        hg_all = sb("hg_all", [cfg.G, HID])
        nc.sync.dma_start(hg_all[:], hg_out[:])
        tp_ps = B_ps[1][:, 0:cfg.G]
        nc.tensor.transpose(tp_ps, hg_all[:], ident[0:cfg.G, 0:cfg.G])
        nc.vector.tensor_copy(hgT_sb[:], tp_ps)
        lg_ps = B_ps[0][0:cfg.G, 512:512 + cfg.C]
        nc.tensor.matmul(lg_ps, hgT_sb[:], Wcls_sb[:], start=True, stop=True)
        nc.vector.tensor_tensor(out=out_sb[:], in0=lg_ps, in1=bcls_sb[:], op=OP.add)
        nc.sync.dma_start(d_out[:], out_sb[:])

    nc.compile()
    return nc


# ---------------------------------------------------------------- entry

_CACHE = {}
LAST_EXEC_NS = None
LAST_RESULTS = None
PROFILE = False


def _get_nc(cfg_key, cfg):
    if cfg_key not in _CACHE:
        _CACHE[cfg_key] = build_nc(cfg)
    return _CACHE[cfg_key]


def kernel(feat, src, dst, etypes, graph_ids, W_e, b_e, W_ih, W_hh, b_ih,
           b_hh, W_cls, b_cls):
    feat = np.asarray(feat, np.float32)
    args = dict(src=np.asarray(src), dst=np.asarray(dst),
                etypes=np.asarray(etypes), graph_ids=np.asarray(graph_ids),
                W_e=np.asarray(W_e, np.float32), b_e=np.asarray(b_e, np.float32),
                W_ih=np.asarray(W_ih, np.float32), W_hh=np.asarray(W_hh, np.float32),
                b_ih=np.asarray(b_ih, np.float32), b_hh=np.asarray(b_hh, np.float32),
                W_cls=np.asarray(W_cls, np.float32), b_cls=np.asarray(b_cls, np.float32))
    cfg = Cfg(**CFG_FULL)
    in_maps = make_plan(feat=feat, cfg=cfg, **args)
    nc = _get_nc("full", cfg)
    res = run_bass_kernel_spmd(nc, in_maps, list(range(cfg.n_cores)),
                               trace=PROFILE)
    global LAST_EXEC_NS, LAST_RESULTS
    LAST_EXEC_NS = res.exec_time_ns
    LAST_RESULTS = res
    return np.asarray(res.results[0]["out"], np.float32)


# revision 10
# speedup vs baseline: 1.0248x; 1.0248x over previous
"""GGNN (gated graph NN) forward on 8 Trainium2 NeuronCores.

Strategy (node-partitioned, SPMD — one Bass program, per-core data):
  - Nodes are permuted and packed into 8 cores x 20 bins x 96 node-column
    slots such that, for every (bin, etype), the number of in-edges is <= 128.
    This makes the aggregation a fixed static structure: one 128-edge tile per
    (etype, bin).
  - h is kept per-core transposed in SBUF ([128 hid, 2048 node-cols]) for all
    dense matmuls; a row-major bf16 copy lives in DRAM (AllGathered each
    step) and is the source for per-edge dma_gather.
  - Per-edge gathers use PREPARE_ONLY descriptor generation + trigger_dma:
    the Q7 (GpSimd) descriptor-gen work (~2.5us per 896-row call, ~98us/step)
    for step s+1 runs during step s's compute/collective phases, off the
    critical path.  Each step fires 4 triggers (one per SWDGE queue).
    Gather buffers rotate over 8 tiles (2-step parity x 4 etypes) so preps
    never wait on current-step consumers.
  - Per step, per etype t: per bin, a one-hot/count matrix S (host-built,
    bf16, exact) scatter-sums the gathered rows into B_t[d, dst] in PSUM via
    TensorE matmuls; then aT += W_t^T @ B_t (float32r).  GRU update runs
    fully on-chip (gates in PSUM, sigmoid/tanh on ACT with per-partition
    bias); graph readout is a one-hot matmul + AllReduce.
"""

import numpy as np
import ml_dtypes

import concourse.bacc as bacc
import concourse.mybir as mybir
import concourse.tile as tile
from concourse.masks import make_identity
from concourse.bass_utils import run_bass_kernel_spmd

BF16_NP = ml_dtypes.bfloat16

F32 = mybir.dt.float32
F32R = mybir.dt.float32r
BF16 = mybir.dt.bfloat16
I16 = mybir.dt.int16
AF = mybir.ActivationFunctionType
OP = mybir.AluOpType

HID = 128
USE_F32R = True  # fp32r (tf32-ish) for W/GRU matmuls
PREP_TRIGGER = False  # SWDGE prepare_only + trigger_dma gathers (slower: prep
                      # desc-gen is ~1.7x the cost and Tile adds a ~1.6us
                      # InstIncSwdgeSem per prep)
SPLIT_AG = False  # two range-AllGathers per step (measured slower: each mesh
                  # collective pays ~11us entry + ~18us events; two serialize)
AG_SPLIT = 1024  # local rows in the first AllGather (512-aligned)


class Cfg:
    def __init__(self, n_cores, shard, bin_cols, n_etypes, n_steps, n_graphs,
                 n_classes, in_dim):
        assert shard % 128 == 0 and shard % bin_cols == 0
        self.n_cores = n_cores
        self.shard = shard                  # node slots per core
        self.bin = bin_cols                 # node columns per bin
        self.bins = shard // bin_cols       # bins per core
        assert self.bins % 2 == 0
        self.half_bins = self.bins // 2
        self.half_cols = self.half_bins * bin_cols   # node cols per B-half
        assert self.half_cols <= 1024
        self.ntot = n_cores * shard
        self.vpad = ((shard + 511) // 512) * 512     # aT psum width
        self.jt = shard // 128              # 128-wide transpose tiles per core
        self.T = n_etypes
        self.steps = n_steps
        self.G = n_graphs
        self.C = n_classes
        self.in_dim = in_dim
        self.idxc = self.bins * 128 // 16   # idx cols per etype
        self.gbins = 7                      # bins per dma_gather call
        self.scols = self.T * self.bins * self.bin   # S cols total
        self.pw = 128                       # gather row elems (bf16)


CFG_FULL = dict(n_cores=8, shard=1920, bin_cols=96, n_etypes=13, n_steps=6,
                n_graphs=64, n_classes=10, in_dim=100)


# ---------------------------------------------------------------- host prep

def _pack_nodes(deg, cfg, rng_order=None):
    """Assign each node to a (global bin, slot) s.t. per-(bin,etype) in-edge
    count <= 128 and per-bin node count <= cfg.bin. Returns slot_of[node]."""
    N = deg.shape[0]
    nbins = cfg.n_cores * cfg.bins
    assert N <= nbins * cfg.bin, "not enough node slots"
    used_e = np.zeros((nbins, cfg.T), np.int64)
    used_s = np.zeros(nbins, np.int64)
    order = np.lexsort((-deg.sum(1), -deg.max(1)))
    bin_of = np.empty(N, np.int64)
    for v in order:
        dv = deg[v]
        ok = (used_s < cfg.bin) & ((used_e + dv) <= 128).all(1)
        if not ok.any():
            raise RuntimeError("bin packing failed; reduce bin_cols")
        cand = np.nonzero(ok)[0]
        load = (used_e[cand] + dv).max(1) * 1.0 + used_s[cand] * 0.01
        b = cand[np.argmin(load)]
        used_e[b] += dv
        bin_of[v] = b
        used_s[b] += 1
    # slots within each bin in node order
    slot_of = np.empty(N, np.int64)
    fill = np.zeros(nbins, np.int64)
    for v in range(N):
        b = bin_of[v]
        core, lb = b // cfg.bins, b % cfg.bins
        slot_of[v] = core * cfg.shard + lb * cfg.bin + fill[b]
        fill[b] += 1
    return slot_of


def make_plan(feat, src, dst, etypes, graph_ids, W_e, b_e, W_ih, W_hh, b_ih,
              b_hh, W_cls, b_cls, cfg):
    N = feat.shape[0]
    T, S_, B_, BINS = cfg.T, cfg.shard, cfg.bin, cfg.bins
    deg = np.zeros((N, T), np.int64)
    np.add.at(deg, (dst, etypes), 1)
    slot_of = _pack_nodes(deg, cfg)

    # --- edge plan ---
    dslot = slot_of[dst]
    sslot = slot_of[src]
    gbin = dslot // B_                       # global bin (slot space is bin-aligned)
    core = dslot // S_
    lbin = gbin - core * BINS
    tile_id = etypes.astype(np.int64) * BINS + lbin      # per-core tile index
    order = np.lexsort((dslot, tile_id, core))
    c_o, t_o, ds_o, ss_o = core[order], tile_id[order], dslot[order], sslot[order]
    # row index within each (core, tile) group
    key = c_o * (T * BINS) + t_o
    boundaries = np.nonzero(np.diff(key))[0] + 1
    starts = np.concatenate([[0], boundaries])
    group_of = np.searchsorted(starts, np.arange(len(key)), side="right") - 1
    row = np.arange(len(key)) - starts[group_of]
    assert row.max() < 128, "edge cap exceeded (packing bug)"

    NC = cfg.n_cores
    # gather-row renumbering: the collective source buffer is laid out as
    # [all cores' rows 0..AG_SPLIT) | all cores' rows [AG_SPLIT..shard)] so
    # each of the two range-AllGathers writes one contiguous slice.
    def g_row(slot):
        c, j = slot // S_, slot % S_
        if not SPLIT_AG:
            return slot
        lo = j < AG_SPLIT
        return np.where(lo, c * AG_SPLIT + j,
                        NC * AG_SPLIT + c * (S_ - AG_SPLIT) + (j - AG_SPLIT))

    S_host = np.zeros((NC, 128, cfg.scols), np.float32)
    idx_lin = np.zeros((NC, T * BINS, 128), np.int64)
    np.add.at(S_host, (c_o, row, t_o * B_ + (ds_o % B_)), 1.0)
    idx_lin[c_o, t_o, row] = g_row(ss_o)

    # idx wrap: per etype block of bins*128 positions -> [16, idxc]
    idx_lin = idx_lin.reshape(NC, T, BINS * 128)
    wrapped = idx_lin.reshape(NC, T, cfg.idxc, 16).transpose(0, 3, 1, 2)
    idx_host = np.tile(wrapped.reshape(NC, 16, T * cfg.idxc), (1, 8, 1))
    idx_host = np.ascontiguousarray(idx_host).astype(np.int16)

    # --- degree matrix (for b_e bias), per core [T, vpad] ---
    D_host = np.zeros((NC, T, cfg.vpad), np.float32)
    np.add.at(D_host, (core, etypes.astype(np.int64), dslot % S_), 1.0)

    # --- graph one-hot, per core [128, jt*G] ---
    node_of_slot = np.full(cfg.ntot, -1, np.int64)
    node_of_slot[slot_of] = np.arange(N)
    G_host = np.zeros((NC, 128, cfg.jt * cfg.G), np.float32)
    for c in range(NC):
        sl = node_of_slot[c * S_:(c + 1) * S_]
        valid = np.nonzero(sl >= 0)[0]
        j, p = valid // 128, valid % 128
        g = graph_ids[sl[valid]]
        G_host[c, p, j * cfg.G + g] = 1.0

    # --- h0 ---
    h0 = np.zeros((cfg.ntot, HID), np.float32)
    h0[slot_of, :cfg.in_dim] = feat
    h0_gr = np.empty_like(h0)
    h0_gr[g_row(np.arange(cfg.ntot))] = h0
    h0_pair = h0_gr.astype(BF16_NP)
    h0T = np.zeros((NC, 128, cfg.vpad), np.float32)
    for c in range(NC):
        h0T[c, :, :S_] = h0[c * S_:(c + 1) * S_].T

    # --- weights ---
    W_host = np.ascontiguousarray(W_e.transpose(1, 0, 2).reshape(128, T * HID))
    WihT = np.ascontiguousarray(W_ih.T)             # [128, 384]
    WhhT = np.ascontiguousarray(W_hh.T)
    bias4 = np.stack([
        b_ih[0:HID] + b_hh[0:HID],                  # r
        b_ih[HID:2 * HID] + b_hh[HID:2 * HID],      # z
        b_ih[2 * HID:],                             # n (input side)
        b_hh[2 * HID:],                             # n (hidden side)
    ], axis=1).astype(np.float32)                   # [128, 4]
    WclsT = np.ascontiguousarray(W_cls.T).astype(np.float32)   # [128, C]
    bclsG = np.tile(b_cls[None, :], (cfg.G, 1)).astype(np.float32)

    in_maps = []
    for c in range(NC):
        in_maps.append({
            "h0_pair": h0_pair,
            "h0T": h0T[c],
            "S": S_host[c].astype(BF16_NP),
            "idx": idx_host[c],
            "D": D_host[c],
            "G": G_host[c],
            "W": W_host.astype(np.float32),
            "Wih": WihT.astype(np.float32),
            "Whh": WhhT.astype(np.float32),
            "be": np.ascontiguousarray(b_e).astype(np.float32),
            "bias4": bias4,
            "Wcls": WclsT,
            "bcls": bclsG,
        })
    return in_maps


# ---------------------------------------------------------------- bass build

def _window_pieces(cfg, b):
    """Split bin b's 96-col window at 512-boundaries of its B-half tile.
    Returns (half, [(b_off, width, s_off), ...]) with b_off relative to the
    half tile."""
    half = b // cfg.half_bins
    start = (b - half * cfg.half_bins) * cfg.bin
    end = start + cfg.bin
    pieces = []
    cur = start
    while cur < end:
        nxt = min(end, (cur // 512 + 1) * 512)
        pieces.append((cur, nxt - cur, cur - start))
        cur = nxt
    return half, pieces


def _wmm_pieces(cfg, half):
    """aT col ranges for the W_t matmul of one B half: split the half's node
    cols at 512-boundaries of the aT tile. Returns [(at_off, width, b_off)]."""
    lo = half * cfg.half_cols
    hi = lo + cfg.half_cols
    out = []
    cur = lo
    while cur < hi:
        nxt = min(hi, (cur // 512 + 1) * 512)
        out.append((cur, nxt - cur, cur - lo))
        cur = nxt
    return out


def build_nc(cfg):
    nc = bacc.Bacc("TRN2", target_bir_lowering=False, debug=False,
                   num_devices=cfg.n_cores, num_swdge_queues=4)
    T, BINS, B_, VP, JT = cfg.T, cfg.bins, cfg.bin, cfg.vpad, cfg.jt
    NCH = 512  # gru chunk
    NQ = 4    # SWDGE queues

    d_pair0 = nc.dram_tensor("h0_pair", [cfg.ntot, cfg.pw], BF16, kind="ExternalInput")
    d_h0T = nc.dram_tensor("h0T", [128, VP], F32, kind="ExternalInput")
    d_S = nc.dram_tensor("S", [128, cfg.scols], BF16, kind="ExternalInput")
    d_idx = nc.dram_tensor("idx", [128, T * cfg.idxc], I16, kind="ExternalInput")
    d_D = nc.dram_tensor("D", [T, VP], F32, kind="ExternalInput")
    d_G = nc.dram_tensor("G", [128, JT * cfg.G], F32, kind="ExternalInput")
    d_W = nc.dram_tensor("W", [128, T * HID], F32, kind="ExternalInput")
    d_Wih = nc.dram_tensor("Wih", [128, 3 * HID], F32, kind="ExternalInput")
    d_Whh = nc.dram_tensor("Whh", [128, 3 * HID], F32, kind="ExternalInput")
    d_be = nc.dram_tensor("be", [T, HID], F32, kind="ExternalInput")
    d_bias4 = nc.dram_tensor("bias4", [128, 4], F32, kind="ExternalInput")
    d_Wcls = nc.dram_tensor("Wcls", [128, cfg.C], F32, kind="ExternalInput")
    d_bcls = nc.dram_tensor("bcls", [cfg.G, cfg.C], F32, kind="ExternalInput")
    d_out = nc.dram_tensor("out", [cfg.G, cfg.C], F32, kind="ExternalOutput")

    # internal dram (collective bounce, double-buffered)
    aspace = "Shared" if cfg.n_cores > 4 else "Local"
    cc_in = [nc.dram_tensor(f"cc_in{i}", [cfg.shard, cfg.pw], BF16)
             for i in range(2)]
    cc_out = [nc.dram_tensor(f"cc_out{i}", [cfg.ntot, cfg.pw], BF16,
                             addr_space=aspace) for i in range(2)]
    hg_in = nc.dram_tensor("hg_in", [cfg.G, HID], F32)
    hg_out = nc.dram_tensor("hg_out", [cfg.G, HID], F32, addr_space=aspace)

    MMDT = F32R if USE_F32R else F32

    with tile.TileContext(nc) as tc:
        def sb(name, shape, dt=F32):
            return nc.alloc_sbuf_tensor(name, list(shape), dt).ap()

        def ps(name, shape, dt=F32):
            return nc.alloc_psum_tensor(name, list(shape), dt).ap()

        S_sb = sb("S_sb", [128, cfg.scols], BF16)
        idx_sb = sb("idx_sb", [128, T * cfg.idxc], I16)
        hT = sb("hT", [128, VP])
        aT_sb = sb("aT_sb", [128, VP], MMDT)
        W_sb = sb("W_sb", [128, T * HID], MMDT)
        Wih_sb = sb("Wih_sb", [128, 3 * HID], MMDT)
        Whh_sb = sb("Whh_sb", [128, 3 * HID], MMDT)
        be_sb = sb("be_sb", [T, HID], MMDT)
        D_sb = sb("D_sb", [T, VP], MMDT)
        bias_sb = sb("bias_sb", [128, 4])
        G_sb = sb("G_sb", [128, JT * cfg.G])
        Wcls_sb = sb("Wcls_sb", [128, cfg.C])
        bcls_sb = sb("bcls_sb", [cfg.G, cfg.C])
        ident = sb("ident", [128, 128])
        h_rows = sb("h_rows", [128, JT * 128])
        pair_sb = sb("pair_sb", [128, JT * cfg.pw], BF16)
        hg_sb = sb("hg_sb", [cfg.G, HID])
        hgT_sb = sb("hgT_sb", [128, cfg.G])
        out_sb = sb("out_sb", [cfg.G, cfg.C])
        hTr = sb("hTr", [128, VP], F32R) if USE_F32R else None

        NGB = 8
        gbuf = [sb(f"gbuf{i}", [128, BINS * cfg.pw], BF16) for i in range(NGB)]
        Bsb = [sb(f"Bsb{i}", [128, cfg.half_cols], MMDT) for i in range(2)]
        # GRU scratch, 2 sets alternating by chunk parity (in-place reuse)
        gsc = [{nm: sb(f"gsc{i}_{nm}", [128, NCH])
                for nm in ("r", "z", "hn", "n")}
               for i in range(2)]

        B_ps = [ps(f"B_ps{i}", [128, 1024]) for i in range(2)]
        aT_ps = ps("aT_ps", [128, VP])

        # ---------------- setup loads ----------------
        if USE_F32R:
            # load fp32 via one staging tile, round via DVE copy into f32r
            stage = sb("stage", [128, VP])
            for dsrc, dst_sb, w in ((d_W, W_sb, T * HID),
                                    (d_Wih, Wih_sb, 3 * HID),
                                    (d_Whh, Whh_sb, 3 * HID)):
                nc.sync.dma_start(stage[:, :w], dsrc[:])
                nc.vector.tensor_copy(dst_sb[:], stage[:, :w])
            stage4 = stage[0:T, 0:HID]
            nc.sync.dma_start(stage4, d_be[:])
            nc.vector.tensor_copy(be_sb[:], stage4)
            nc.sync.dma_start(stage[0:T, :], d_D[:])
            nc.vector.tensor_copy(D_sb[:], stage[0:T, :])
        else:
            nc.sync.dma_start(W_sb[:], d_W[:])
            nc.sync.dma_start(Wih_sb[:], d_Wih[:])
            nc.sync.dma_start(Whh_sb[:], d_Whh[:])
            nc.sync.dma_start(be_sb[:], d_be[:])
            nc.sync.dma_start(D_sb[:], d_D[:])
        nc.sync.dma_start(idx_sb[:], d_idx[:])
        SC = BINS * B_
        for t in range(T):
            nc.sync.dma_start(S_sb[:, t * SC:(t + 1) * SC],
                              d_S[:, t * SC:(t + 1) * SC])
        nc.sync.dma_start(hT[:], d_h0T[:])
        nc.sync.dma_start(bias_sb[:], d_bias4[:])
        nc.sync.dma_start(G_sb[:], d_G[:])
        nc.sync.dma_start(Wcls_sb[:], d_Wcls[:])
        nc.sync.dma_start(bcls_sb[:], d_bcls[:])
        make_identity(nc, ident[:])

        # ---------------- gather plan ----------------
        GB = cfg.gbins
        CALLS = [(t, b0, min(GB, BINS - b0))
                 for t in range(T) for b0 in range(0, BINS, GB)]

        def g3_of(s, t):
            g = gbuf[(s % 2) * 4 + (t % 4)]
            return g[:].rearrange("p (b d) -> p b d", d=cfg.pw)

        def emit_preps(s):
            """PREPARE_ONLY descriptor gen for all of step s's gathers,
            grouped by queue so each queue's trigger can fire as soon as its
            own preps are written."""
            pair_src = d_pair0 if s == 0 else cc_out[s % 2]
            sems = [nc.alloc_semaphore(f"gsem_{s}_{q}") for q in range(NQ)]
            for q in range(NQ):
                for i, (t, b0, nb) in enumerate(CALLS):
                    if i % NQ != q:
                        continue
                    nc.gpsimd.dma_gather(
                        g3_of(s, t)[:, b0:b0 + nb, :], pair_src[:],
                        idx_sb[:, t * cfg.idxc + b0 * 8:
                               t * cfg.idxc + (b0 + nb) * 8],
                        nb * 128, nb * 128, cfg.pw,
                        prepare_only=True, sem=sems[q], queue_num=q)

        if PREP_TRIGGER:
            emit_preps(0)

        # ---------------- steps ----------------
        gq = [0]  # rotating SWDGE queue for non-prep gathers
        for s in range(cfg.steps):
            pair_src = d_pair0 if s == 0 else cc_out[s % 2]

            if PREP_TRIGGER:
                for q in range(NQ):
                    nc.gpsimd.trigger_dma(count=None, queue_num=q)

            # deg * b_e bias: aT = be^T @ D  (start=True covers all of aT)
            for c0 in range(0, VP, 512):
                nc.tensor.matmul(aT_ps[:, c0:c0 + 512], be_sb[:],
                                 D_sb[:, c0:c0 + 512], start=True, stop=False)

            for t in range(T):
                g3 = g3_of(s, t)
                if not PREP_TRIGGER:
                    for b0 in range(0, BINS, GB):
                        nb = min(GB, BINS - b0)
                        nc.gpsimd.dma_gather(
                            g3[:, b0:b0 + nb, :], pair_src[:],
                            idx_sb[:, t * cfg.idxc + b0 * 8:
                                   t * cfg.idxc + (b0 + nb) * 8],
                            nb * 128, nb * 128, cfg.pw,
                            queue_num=gq[0] % NQ)
                        gq[0] += 1
                for half in range(2):
                    Bp = B_ps[half]
                    # flat entry list: (bank, b_off, w, s_col)
                    entries = []
                    for bi in range(cfg.half_bins):
                        b = half * cfg.half_bins + bi
                        _, pieces = _window_pieces(cfg, b)
                        sbase = (t * BINS + b) * B_
                        for (b_off, w, s_off) in pieces:
                            entries.append((b_off // 512, b_off, w,
                                            sbase + s_off))
                    first_of = {}
                    last_of = {}
                    for i, e in enumerate(entries):
                        first_of.setdefault(e[0], i)
                        last_of[e[0]] = i
                    for i, (bank, b_off, w, s_col) in enumerate(entries):
                        nc.tensor.matmul(
                            Bp[:, b_off:b_off + w],
                            g3[:, (b_off + half * cfg.half_cols) // B_, 0:HID],
                            S_sb[:, s_col:s_col + w],
                            start=(first_of[bank] == i),
                            stop=(last_of[bank] == i))
                    # PSUM -> SBUF (rounds to f32r when enabled)
                    if (t * 2 + half) % 2:
                        nc.scalar.activation(Bsb[half][:, :], Bp[:, :cfg.half_cols],
                                             AF.Identity)
                    else:
                        nc.vector.tensor_copy(Bsb[half][:, :], Bp[:, :cfg.half_cols])
                    # aT += W_t^T @ B_half.  stop=True only on the final
                    # accumulation touching each aT bank (t==T-1; for banks
                    # shared by both halves, only half 1's piece closes it).
                    lhsW = W_sb[:, t * HID:(t + 1) * HID]
                    for (at_off, w, b_off) in _wmm_pieces(cfg, half):
                        bank_end = (at_off + w - 1) // 512
                        shared = (cfg.half_cols % 512) != 0 and \
                            bank_end == cfg.half_cols // 512
                        is_stop = (t == T - 1) and not (half == 0 and shared)
                        nc.tensor.matmul(aT_ps[:, at_off:at_off + w], lhsW,
                                         Bsb[half][:, b_off:b_off + w],
                                         start=False, stop=is_stop)

            gru_rhs_h = hTr if USE_F32R else hT

            # aT psum -> sbuf (chunked; must all finish before odd GRU
            # chunks reuse aT_ps banks for gates)
            for c0 in range(0, VP, NCH):
                nc.scalar.activation(aT_sb[:, c0:c0 + NCH],
                                     aT_ps[:, c0:c0 + NCH], AF.Identity)
                if USE_F32R:
                    nc.vector.tensor_copy(hTr[:, c0:c0 + NCH], hT[:, c0:c0 + NCH])

            # ---------------- GRU ----------------
            for ci, c0 in enumerate(range(0, VP, NCH)):
                if ci % 2 == 0:
                    gA, gB = B_ps[0], B_ps[1]
                else:
                    gA, gB = aT_ps[:, 0:2 * NCH], aT_ps[:, 2 * NCH:4 * NCH]
                r_ps, z_ps = gA[:, 0:NCH], gA[:, NCH:2 * NCH]
                ni_ps, nh_ps = gB[:, 0:NCH], gB[:, NCH:2 * NCH]
                a_c = aT_sb[:, c0:c0 + NCH]
                h_c = gru_rhs_h[:, c0:c0 + NCH]
                nc.tensor.matmul(r_ps, Wih_sb[:, 0:HID], a_c, start=True, stop=False)
                nc.tensor.matmul(r_ps, Whh_sb[:, 0:HID], h_c, start=False, stop=True)
                nc.tensor.matmul(z_ps, Wih_sb[:, HID:2 * HID], a_c, start=True, stop=False)
                nc.tensor.matmul(z_ps, Whh_sb[:, HID:2 * HID], h_c, start=False, stop=True)
                nc.tensor.matmul(ni_ps, Wih_sb[:, 2 * HID:3 * HID], a_c, start=True, stop=True)
                nc.tensor.matmul(nh_ps, Whh_sb[:, 2 * HID:3 * HID], h_c, start=True, stop=True)

                sc = gsc[ci % 2]
                r_sb, z_sb, hn_sb, n_sb = sc["r"], sc["z"], sc["hn"], sc["n"]
                nc.scalar.activation(r_sb[:], r_ps, AF.Sigmoid, bias=bias_sb[:, 0:1])
                nc.scalar.activation(z_sb[:], z_ps, AF.Sigmoid, bias=bias_sb[:, 1:2])
                nc.scalar.activation(hn_sb[:], nh_ps, AF.Identity, bias=bias_sb[:, 3:4])
                nc.vector.tensor_tensor(out=r_sb[:], in0=r_sb[:], in1=hn_sb[:], op=OP.mult)
                nc.vector.tensor_tensor(out=r_sb[:], in0=r_sb[:], in1=ni_ps, op=OP.add)
                nc.scalar.activation(n_sb[:], r_sb[:], AF.Tanh, bias=bias_sb[:, 2:3])
                nc.vector.tensor_tensor(out=hn_sb[:], in0=hT[:, c0:c0 + NCH], in1=n_sb[:], op=OP.subtract)
                nc.vector.tensor_tensor(out=hn_sb[:], in0=hn_sb[:], in1=z_sb[:], op=OP.mult)
                nc.vector.tensor_tensor(out=hT[:, c0:c0 + NCH], in0=hn_sb[:], in1=n_sb[:], op=OP.add)

            # ------- transpose h -> rows; pack bf16 + DMA per 512-range ---
            tp_slots = [B_ps[0][:, 0:128], B_ps[1][:, 0:128],
                        aT_ps[:, 0:128]]
            if VP >= 2048:
                tp_slots.append(aT_ps[:, 1024:1152])
            hr3 = h_rows[:].rearrange("p (j d) -> p j d", d=128)
            pr3 = pair_sb[:].rearrange("p (j d) -> p j d", d=cfg.pw)
            dst = cc_in[(s + 1) % 2]
            dst3 = dst[:].rearrange("(j p) d -> p j d", p=128)
            ccd = cc_out[(s + 1) % 2]
            rgrp = [list(range(cfg.n_cores))]
            JA = AG_SPLIT // 128  # j-tiles in the first AllGather
            j_done = 0
            for j in range(JT):
                tp = tp_slots[j % len(tp_slots)]
                nc.tensor.transpose(tp, hT[:, j * 128:(j + 1) * 128], ident[:])
                if j % 2:
                    nc.scalar.activation(h_rows[:, j * 128:(j + 1) * 128], tp, AF.Identity)
                else:
                    nc.vector.tensor_copy(h_rows[:, j * 128:(j + 1) * 128], tp)
                rng_end = (j + 1) * 128
                if s < cfg.steps - 1 and (rng_end % NCH == 0 or j == JT - 1):
                    j0, j1 = j_done, j + 1
                    j_done = j + 1
                    nc.scalar.activation(pr3[:, j0:j1, 0:HID],
                                         hr3[:, j0:j1, :], AF.Identity)
                    nc.sync.dma_start(dst3[:, j0:j1, :], pr3[:, j0:j1, :])
                if s < cfg.steps - 1 and SPLIT_AG and j + 1 == JA:
                    nc.gpsimd.collective_compute(
                        "AllGather", OP.bypass,
                        ins=[dst[0:AG_SPLIT, :]],
                        outs=[ccd[0:cfg.n_cores * AG_SPLIT, :]],
                        replica_groups=rgrp)
            if s < cfg.steps - 1:
                if SPLIT_AG:
                    nc.gpsimd.collective_compute(
                        "AllGather", OP.bypass,
                        ins=[dst[AG_SPLIT:cfg.shard, :]],
                        outs=[ccd[cfg.n_cores * AG_SPLIT:cfg.ntot, :]],
                        replica_groups=rgrp)
                else:
                    nc.gpsimd.collective_compute(
                        "AllGather", OP.bypass,
                        ins=[dst[:]], outs=[ccd[:]],
                        replica_groups=rgrp)
                if PREP_TRIGGER:
                    emit_preps(s + 1)

        # ---------------- readout ----------------
        hg_ps = B_ps[0][0:cfg.G, 0:HID]
        for j in range(JT):
            nc.tensor.matmul(hg_ps, G_sb[:, j * cfg.G:(j + 1) * cfg.G],
                             h_rows[:, j * 128:(j + 1) * 128],
                             start=(j == 0), stop=(j == JT - 1))
        nc.scalar.activation(hg_sb[:], hg_ps, AF.Identity)
        nc.sync.dma_start(hg_in[:], hg_sb[:])
        nc.gpsimd.collective_compute(
            "AllReduce", OP.add, ins=[hg_in[:]], outs=[hg_out[:]],
            replica_groups=[list(range(cfg.n_cores))])
        hg_all = sb("hg_all", [cfg.G, HID])
        nc.sync.dma_start(hg_all[:], hg_out[:])
        tp_ps = B_ps[1][:, 0:cfg.G]
        nc.tensor.transpose(tp_ps, hg_all[:], ident[0:cfg.G, 0:cfg.G])
        nc.vector.tensor_copy(hgT_sb[:], tp_ps)
        lg_ps = B_ps[0][0:cfg.G, 512:512 + cfg.C]
        nc.tensor.matmul(lg_ps, hgT_sb[:], Wcls_sb[:], start=True, stop=True)
        nc.vector.tensor_tensor(out=out_sb[:], in0=lg_ps, in1=bcls_sb[:], op=OP.add)
        nc.sync.dma_start(d_out[:], out_sb[:])

    nc.compile()
    return nc


# ---------------------------------------------------------------- entry

_CACHE = {}
LAST_EXEC_NS = None
LAST_RESULTS = None
PROFILE = False


def _get_nc(cfg_key, cfg):
    if cfg_key not in _CACHE:
        _CACHE[cfg_key] = build_nc(cfg)
    return _CACHE[cfg_key]


def kernel(feat, src, dst, etypes, graph_ids, W_e, b_e, W_ih, W_hh, b_ih,
           b_hh, W_cls, b_cls):
    feat = np.asarray(feat, np.float32)
    args = dict(src=np.asarray(src), dst=np.asarray(dst),
                etypes=np.asarray(etypes), graph_ids=np.asarray(graph_ids),
                W_e=np.asarray(W_e, np.float32), b_e=np.asarray(b_e, np.float32),
                W_ih=np.asarray(W_ih, np.float32), W_hh=np.asarray(W_hh, np.float32),
                b_ih=np.asarray(b_ih, np.float32), b_hh=np.asarray(b_hh, np.float32),
                W_cls=np.asarray(W_cls, np.float32), b_cls=np.asarray(b_cls, np.float32))
    cfg = Cfg(**CFG_FULL)
    in_maps = make_plan(feat=feat, cfg=cfg, **args)
    nc = _get_nc("full", cfg)
    res = run_bass_kernel_spmd(nc, in_maps, list(range(cfg.n_cores)),
                               trace=PROFILE)
    global LAST_EXEC_NS, LAST_RESULTS
    LAST_EXEC_NS = res.exec_time_ns
    LAST_RESULTS = res
    return np.asarray(res.results[0]["out"], np.float32)
